# revision 19
# baseline (speedup 1.0000x reference)
"""Binarized linear layer (BLinear) Trainium2 kernel, v2.

Computes y = sign(x) @ sign(W).T + b for x [8192, 2048] f32, W [2048, 2048] f32,
b [2048] f32. Data-parallel across 8 NeuronCores (1024 tokens per core, W
replicated).

Math notes (all exact => bit-exact vs the fp32 reference):
 - sign() in {-1, 0, +1} is exact in bf16/fp8e4; TensorE accumulates fp32 in
   PSUM; sums of +-1 over K=2048 are exact integers << 2^24.
 - x and W are staged to DRAM as bf16 (host cast). bf16 keeps fp32's exponent
   range, so the cast preserves sign()/zeroness for every fp32 input.
 - y is computed TRANSPOSED on device (yT [2048 o, 1024 t] fp16; integer sums
   with |y| <= 2048 are exact in fp16; bias added on-device from f32 PSUM
   before the cast) and un-transposed/widened on the host.

v2 structure (vs v1's 96.4us):
 - Stationary operand is W (lhsT = wb tile [128ki, 2ko, 128o]); each
   LDWEIGHTS is reused by 2 streaming matmuls (rhs = xb [128ki, 2ko, 512t]),
   so the DoubleRow 256-col weight load always hides under ~480ns of
   streaming.
 - The benchmark loop body holds TWO unrolled iterations and every operand
   tile pool has bufs=2, so iteration i+1's DMA-transpose + sign prep runs
   concurrently with iteration i's matmuls (cross-iteration software
   pipelining; v1 serialized prep behind the previous iteration's matmuls).
 - sign() work is split: ScalarE does most chunks natively, VectorE does 3
   W chunks via (min(v*2^126, 1) then max(.,-1)) two-op form (exact for all
   |v| >= 2^-126; inputs here have |v| >= ~2^-28).
 - PSUM eviction (bias add + fp16 cast) all on VectorE tensor_scalar_add
   with a per-partition (=per-o) bias AP.
"""

import numpy as np

N_CORES = 8
TOKENS = 8192
D_IN = 2048
D_OUT = 2048
T_CORE = TOKENS // N_CORES  # 1024 tokens per core

P = 128
KO = D_IN // P          # 16 contraction chunks of 128
KP = KO // 2            # 8 DoubleRow K-pairs (256 per matmul)
NB = 512                # matmul moving free dim / PSUM bank (fp32)
TH = T_CORE // NB       # 2 token halves of 512
OC = D_OUT // P         # 16 out-feature tiles of 128
WCH = 8                 # W prep chunks (256 o-rows each)
WCO = D_OUT // WCH      # 256 o per W chunk

_CACHE = {}
LAST_RESULT = None


def _build_bass(loop_n=1, phase="all", mm_struct="wstat"):
    import concourse.mybir as mybir
    import concourse.tile as tile
    from concourse import bacc
    from concourse.bass import ts

    nc = bacc.Bacc(
        "TRN2",
        target_bir_lowering=False,
        debug=False,
        enable_asserts=False,
    )

    f32 = mybir.dt.float32
    bf16 = mybir.dt.bfloat16
    fp16 = mybir.dt.float16
    fp8 = mybir.dt.float8e4

    x_d = nc.dram_tensor("x", [T_CORE, D_IN], bf16, kind="ExternalInput")
    w_d = nc.dram_tensor("W", [D_OUT, D_IN], bf16, kind="ExternalInput")
    b_d = nc.dram_tensor("bt", [P, OC], f32, kind="ExternalInput")
    y_d = nc.dram_tensor("yT", [D_OUT, T_CORE], fp16, kind="ExternalOutput")

    x_ap = x_d.ap()
    w_ap = w_d.ap()
    b_ap = b_d.ap()
    y_ap = y_d.ap()

    unroll = 4 if loop_n > 1 else 1

    with tile.TileContext(nc) as tc:
        with (
            tc.tile_pool(name="ops", bufs=2) as ops,
            tc.tile_pool(name="xstage", bufs=2) as xstage,
            tc.tile_pool(name="wstage", bufs=3) as wstage,
            tc.tile_pool(name="dvetmp", bufs=2) as dvetmp,
            tc.tile_pool(name="outp", bufs=4) as out_pool,
            tc.tile_pool(name="psum", bufs=8, space="PSUM") as psum_pool,
        ):
            ev_i = 0

            def body_one(u):
                nonlocal ev_i
                # --- operand tiles for this (unrolled) iteration ---
                xb = [ops.tile([P, KO, NB], fp8, name=f"xb{h}") for h in range(TH)]
                if mm_struct == "wstat":
                    wb = [ops.tile([P, KO, WCO], fp8, name=f"wb{c}") for c in range(WCH)]
                else:
                    wb = [ops.tile([P, KO, NB], fp8, name=f"wb{c}") for c in range(4)]
                bias = ops.tile([P, OC], f32, name="bias")

                if phase == "mm":
                    # timing-only build: tiny slice writes allocate the tiles
                    # (full contents are garbage; numerics unused)
                    for t_ in xb + wb:
                        nc.gpsimd.memset(t_[:, 0, 0:1], 1.0)
                    nc.gpsimd.memset(bias[:, 0:1], 0.0)
                else:
                    # --- prep: DMA-transpose from DRAM (bf16) + sign -> fp8 ---
                    nc.gpsimd.dma_start(bias[:], b_ap[:, :])

                    def sign_act(dst, src):
                        nc.scalar.sign(dst, src)

                    def sign_2op(eng, dst, src, shape):
                        # exact sign for all |v| >= 2^-126 (incl. v == 0)
                        tmp = dvetmp.tile(shape, bf16, name="dvetmp")
                        eng.tensor_scalar(
                            tmp[:], src, 2.0 ** 126, 1.0,
                            mybir.AluOpType.mult, mybir.AluOpType.min,
                        )
                        eng.tensor_scalar_max(dst, tmp[:], -1.0)

                    def prep_x(h):
                        st = xstage.tile([P, KO, NB], bf16, name="xst")
                        nc.sync.dma_start_transpose(st[:], x_ap[ts(h, NB), :])
                        if phase == "dma":
                            nc.vector.tensor_copy(xb[h][:, 0, 0:1], st[:, 0, 0:1])
                            return
                        sign_act(xb[h][:], st[:])

                    def prep_w(c):
                        st = wstage.tile([P, KO, WCO], bf16, name="wst")
                        nc.sync.dma_start_transpose(st[:], w_ap[ts(c, WCO), :])
                        if phase == "dma":
                            nc.vector.tensor_copy(wb[c][:, 0, 0:1], st[:, 0, 0:1])
                            return
                        if c in (0, 2, 4):
                            sign_2op(nc.vector, wb[c][:], st[:], [P, KO, WCO])
                        elif c in (6, 7):
                            sign_2op(nc.gpsimd, wb[c][:], st[:], [P, KO, WCO])
                        else:
                            sign_act(wb[c][:], st[:])

                    prep_x(0)
                    prep_w(0)
                    prep_x(1)
                    for c in range(1, WCH):
                        prep_w(c)

                if phase in ("prep", "dma"):
                    # tiny consumers so prep work can't be dead-code'd away
                    o_sb = out_pool.tile([P, NB], fp16, tag="osb", name="o_sb")
                    for i, t_ in enumerate(xb + wb):
                        nc.vector.tensor_copy(
                            o_sb[:, i : i + 1], t_[:, 0, 0:1]
                        )
                    nc.scalar.dma_start(y_ap[ts(0, P), ts(0, NB)], o_sb[:])
                    return

                if mm_struct in ("wstat", "wsame"):
                    # th OUTER: the first 128 matmuls consume only xb[0], so
                    # PE starts as soon as x half 0 + W chunk 0 are signed.
                    for th in range(TH):
                        for oc in range(OC):
                            c, lo = divmod(oc, 2)
                            if mm_struct == "wsame":
                                c, lo = 0, 0  # fixed stationary: LDW-elision probe
                            psum = psum_pool.tile([P, NB], f32, tag="psum", name="psum")
                            for kp in range(KP):
                                kp_ = 0 if mm_struct == "wsame" else kp
                                nc.tensor.matmul(
                                    psum[:],
                                    lhsT=wb[c][:, 2 * kp_ : 2 * kp_ + 2, ts(lo, P)],
                                    rhs=xb[th][:, 2 * kp : 2 * kp + 2, :],
                                    perf_mode=mybir.MatmulPerfMode.DoubleRow,
                                    start=(kp == 0),
                                    stop=(kp == KP - 1),
                                )
                            o_sb = out_pool.tile([P, NB], fp16, tag="osb", name="o_sb")
                            # evictions alternate DVE / ACT for engine balance
                            if ev_i % 2 == 0:
                                nc.vector.tensor_scalar_add(
                                    o_sb[:], psum[:], bias[:, oc : oc + 1]
                                )
                            else:
                                nc.scalar.activation(
                                    o_sb[:],
                                    psum[:],
                                    mybir.ActivationFunctionType.Identity,
                                    bias=bias[:, oc : oc + 1],
                                )
                            ev_i += 1
                            # stores issue from the ACT sequencer (also HWDGE) so
                            # they don't queue behind SP's transpose stream
                            nc.scalar.dma_start(y_ap[ts(oc, P), ts(th, NB)], o_sb[:])
                else:
                    # xstat: v1-style — stationary x token-tile, moving W bank
                    # [128, 2, 512]; timing-only build (phase="mm").
                    assert phase == "mm"
                    for ob in range(4):
                        for tt in range(8):
                            th, tl = divmod(tt, 4)
                            psum = psum_pool.tile([P, NB], f32, tag="psum", name="psum")
                            for kp in range(KP):
                                nc.tensor.matmul(
                                    psum[:],
                                    lhsT=xb[th][:, 2 * kp : 2 * kp + 2, ts(tl, P)],
                                    rhs=wb[ob][:, 2 * kp : 2 * kp + 2, :],
                                    perf_mode=mybir.MatmulPerfMode.DoubleRow,
                                    start=(kp == 0),
                                    stop=(kp == KP - 1),
                                )
                            o_sb = out_pool.tile([P, NB], fp16, tag="osb", name="o_sb")
                            nc.vector.tensor_scalar_add(
                                o_sb[:], psum[:], bias[:, 0:1]
                            )
                            # timing-only: yT is [2048, 1024]; write any
                            # distinct in-range region per (ob, tt)
                            nc.scalar.dma_start(
                                y_ap[ts(2 * ob + (tt % 2), P), ts(tt // 4, NB)],
                                o_sb[:],
                            )

            def body():
                for u in range(unroll):
                    body_one(u)

            if loop_n > 1:
                assert loop_n % unroll == 0
                # staggered_reset=False: the 4-stage adjacency gating couples
                # PE progress to the DMA stream (measured +40us/iter); a full
                # back-edge barrier per 4-copy body costs only ~0.5us/iter
                # and lets copy N+1's prep overlap copy N's matmuls freely.
                with tc.For_i(
                    0,
                    loop_n // unroll,
                    1,
                    hint_engines=(mybir.EngineType.PE,),
                    staggered_reset=False,
                ):
                    body()
            else:
                body()

    nc.compile()
    return nc


def _get_nc():
    if "nc" not in _CACHE:
        _CACHE["nc"] = _build_bass()
    return _CACHE["nc"]


def _host_inputs(inputs):
    import ml_dtypes

    x = np.asarray(inputs["x"], dtype=np.float32)
    W = np.asarray(inputs["W"], dtype=np.float32)
    b = np.ascontiguousarray(np.asarray(inputs["b"], dtype=np.float32))

    # bf16 staging: sign-preserving (bf16 keeps fp32's exponent range)
    x16 = np.ascontiguousarray(x.astype(ml_dtypes.bfloat16))
    W16 = np.ascontiguousarray(W.astype(ml_dtypes.bfloat16))
    # bias transposed to per-partition layout: bt[p, c] = b[c*128 + p]
    bt = np.ascontiguousarray(b.reshape(OC, P).T)
    return x16, W16, bt


def kernel(**inputs):
    global LAST_RESULT

    from concourse.bass_utils import run_bass_kernel_spmd

    x16, W16, bt = _host_inputs(inputs)

    nc = _get_nc()
    in_maps = [
        {
            "x": np.ascontiguousarray(x16[c * T_CORE : (c + 1) * T_CORE]),
            "W": W16,
            "bt": bt,
        }
        for c in range(N_CORES)
    ]
    res = run_bass_kernel_spmd(nc, in_maps, core_ids=list(range(N_CORES)))
    LAST_RESULT = res
    # un-transpose per-core yT [2048, 1024] -> y [1024, 2048]; widen to f32
    y = np.concatenate(
        [np.ascontiguousarray(r["yT"].T) for r in res.results], axis=0
    )
    return y.astype(np.float32)


# revision 30
# speedup vs baseline: 3.3954x; 3.3954x over previous
"""Binarized linear layer (BLinear) Trainium2 kernel, v2.

Computes y = sign(x) @ sign(W).T + b for x [8192, 2048] f32, W [2048, 2048] f32,
b [2048] f32. Data-parallel across 8 NeuronCores (1024 tokens per core, W
replicated).

Math notes (all exact => bit-exact vs the fp32 reference):
 - sign() in {-1, 0, +1} is exact in bf16/fp8e4; TensorE accumulates fp32 in
   PSUM; sums of +-1 over K=2048 are exact integers << 2^24.
 - x and W are staged to DRAM as bf16 (host cast). bf16 keeps fp32's exponent
   range, so the cast preserves sign()/zeroness for every fp32 input.
 - y is computed TRANSPOSED on device (yT [2048 o, 1024 t] fp16; integer sums
   with |y| <= 2048 are exact in fp16; bias added on-device from f32 PSUM
   before the cast) and un-transposed/widened on the host.

v2 structure (vs v1's 96.4us):
 - Stationary operand is W (lhsT = wb tile [128ki, 2ko, 128o]); each
   LDWEIGHTS is reused by 2 streaming matmuls (rhs = xb [128ki, 2ko, 512t]),
   so the DoubleRow 256-col weight load always hides under ~480ns of
   streaming.
 - The benchmark loop body holds TWO unrolled iterations and every operand
   tile pool has bufs=2, so iteration i+1's DMA-transpose + sign prep runs
   concurrently with iteration i's matmuls (cross-iteration software
   pipelining; v1 serialized prep behind the previous iteration's matmuls).
 - sign() work is split: ScalarE does most chunks natively, VectorE does 3
   W chunks via (min(v*2^126, 1) then max(.,-1)) two-op form (exact for all
   |v| >= 2^-126; inputs here have |v| >= ~2^-28).
 - PSUM eviction (bias add + fp16 cast) all on VectorE tensor_scalar_add
   with a per-partition (=per-o) bias AP.
"""

import numpy as np

N_CORES = 8
TOKENS = 8192
D_IN = 2048
D_OUT = 2048
T_CORE = TOKENS // N_CORES  # 1024 tokens per core

P = 128
KO = D_IN // P          # 16 contraction chunks of 128
KP = KO // 2            # 8 DoubleRow K-pairs (256 per matmul)
NB = 512                # matmul moving free dim / PSUM bank (fp32)
TH = T_CORE // NB       # 2 token halves of 512
OC = D_OUT // P         # 16 out-feature tiles of 128
WCH = 8                 # W prep chunks (256 o-rows each)
WCO = D_OUT // WCH      # 256 o per W chunk

_CACHE = {}
LAST_RESULT = None


def _build_bass(loop_n=1, phase="all", mm_struct="wstat", sign_gps=False,
                loop_mode="staggered4"):
    import concourse.mybir as mybir
    import concourse.tile as tile
    from concourse import bacc
    from concourse.bass import ts

    nc = bacc.Bacc(
        "TRN2",
        target_bir_lowering=False,
        debug=False,
        enable_asserts=False,
    )

    f32 = mybir.dt.float32
    bf16 = mybir.dt.bfloat16
    fp16 = mybir.dt.float16
    fp8 = mybir.dt.float8e4

    # xp/Wp are host-permuted to the contraction-major SBUF layout
    # [ki, ko, t|o] (bit-exact bf16 values, pure layout staging), so the
    # device does plain contiguous DMA loads — no xbar DMA-transpose.
    x_d = nc.dram_tensor("xp", [P, KO, T_CORE], bf16, kind="ExternalInput")
    w_d = nc.dram_tensor("Wp", [P, KO, D_OUT], bf16, kind="ExternalInput")
    b_d = nc.dram_tensor("bt", [P, OC], f32, kind="ExternalInput")
    y_d = nc.dram_tensor("yT", [D_OUT, T_CORE], fp16, kind="ExternalOutput")

    x_ap = x_d.ap()
    w_ap = w_d.ap()
    b_ap = b_d.ap()
    y_ap = y_d.ap()

    unroll = 4 if loop_n > 1 else 1

    with tile.TileContext(nc) as tc:
        with (
            tc.tile_pool(name="ops", bufs=2) as ops,
            tc.tile_pool(name="xstage", bufs=2) as xstage,
            tc.tile_pool(name="wstage", bufs=4) as wstage,
            tc.tile_pool(name="dvetmp", bufs=2) as dvetmp,
            tc.tile_pool(name="outp", bufs=4) as out_pool,
            tc.tile_pool(name="psum", bufs=8, space="PSUM") as psum_pool,
        ):
            ev_i = 0

            def body_one(u):
                nonlocal ev_i
                # --- operand tiles for this (unrolled) iteration ---
                xb = [ops.tile([P, KO, NB], fp8, name=f"xb{h}") for h in range(TH)]
                if mm_struct == "wstat":
                    wb = [ops.tile([P, KO, WCO], fp8, name=f"wb{c}") for c in range(WCH)]
                else:
                    wb = [ops.tile([P, KO, NB], fp8, name=f"wb{c}") for c in range(4)]
                bias = ops.tile([P, OC], f32, name="bias")

                if phase == "mm":
                    # timing-only build: tiny slice writes allocate the tiles
                    # (full contents are garbage; numerics unused)
                    for t_ in xb + wb:
                        nc.gpsimd.memset(t_[:, 0, 0:1], 1.0)
                    nc.gpsimd.memset(bias[:, 0:1], 0.0)
                else:
                    # --- prep: DMA-transpose from DRAM (bf16) + sign -> fp8 ---
                    nc.gpsimd.dma_start(bias[:], b_ap[:, :])

                    def sign_act(dst, src):
                        nc.scalar.sign(dst, src)

                    def sign_2op(eng, dst, src, shape):
                        # exact sign for all |v| >= 2^-126 (incl. v == 0)
                        tmp = dvetmp.tile(shape, bf16, name="dvetmp")
                        eng.tensor_scalar(
                            tmp[:], src, 2.0 ** 126, 1.0,
                            mybir.AluOpType.mult, mybir.AluOpType.min,
                        )
                        eng.tensor_scalar_max(dst, tmp[:], -1.0)

                    def prep_x(h):
                        st = xstage.tile([P, KO, NB], bf16, name="xst")
                        nc.sync.dma_start(st[:], x_ap[:, :, ts(h, NB)])
                        if phase == "dma":
                            nc.vector.tensor_copy(xb[h][:, 0, 0:1], st[:, 0, 0:1])
                            return
                        sign_act(xb[h][:], st[:])

                    def prep_w(c):
                        st = wstage.tile([P, KO, WCO], bf16, name="wst")
                        nc.sync.dma_start(st[:], w_ap[:, :, ts(c, WCO)])
                        if phase == "dma":
                            nc.vector.tensor_copy(wb[c][:, 0, 0:1], st[:, 0, 0:1])
                            return
                        # DVE gets the LATE-consumed chunks (w5..w7): its queue
                        # drains copy u's evictions first, so u+1's DVE signs
                        # land mid-mm — early chunks must come from ACT, whose
                        # queue holds only signs and drains well before u ends.
                        if c >= 5:
                            sign_2op(nc.vector, wb[c][:], st[:], [P, KO, WCO])
                        else:
                            sign_act(wb[c][:], st[:])

                    prep_x(0)
                    prep_w(0)
                    prep_x(1)
                    for c in range(1, WCH):
                        prep_w(c)

                if phase in ("prep", "dma"):
                    # tiny consumers so prep work can't be dead-code'd away
                    o_sb = out_pool.tile([P, NB], fp16, tag="osb", name="o_sb")
                    for i, t_ in enumerate(xb + wb):
                        nc.vector.tensor_copy(
                            o_sb[:, i : i + 1], t_[:, 0, 0:1]
                        )
                    nc.scalar.dma_start(y_ap[ts(0, P), ts(0, NB)], o_sb[:])
                    return

                if mm_struct in ("wstat", "wsame"):
                    # th OUTER: the first 128 matmuls consume only xb[0], so
                    # PE starts as soon as x half 0 + W chunk 0 are signed.
                    for th in range(TH):
                        for oc in range(OC):
                            c, lo = divmod(oc, 2)
                            if mm_struct == "wsame":
                                c, lo = 0, 0  # fixed stationary: LDW-elision probe
                            psum = psum_pool.tile([P, NB], f32, tag="psum", name="psum")
                            for kp in range(KP):
                                kp_ = 0 if mm_struct == "wsame" else kp
                                nc.tensor.matmul(
                                    psum[:],
                                    lhsT=wb[c][:, 2 * kp_ : 2 * kp_ + 2, ts(lo, P)],
                                    rhs=xb[th][:, 2 * kp : 2 * kp + 2, :],
                                    perf_mode=mybir.MatmulPerfMode.DoubleRow,
                                    start=(kp == 0),
                                    stop=(kp == KP - 1),
                                )
                            o_sb = out_pool.tile([P, NB], fp16, tag="osb", name="o_sb")
                            # evictions on DVE (ACT Identity would force
                            # activation-table switches against the Sign ops)
                            nc.vector.tensor_scalar_add(
                                o_sb[:], psum[:], bias[:, oc : oc + 1]
                            )
                            ev_i += 1
                            # stores issue via GPS/SWDGE: the store issues are
                            # paced by the mm span, and any engine that also
                            # did sign work would FIFO-block the next copy's
                            # signs behind them. GPS does only stores + bias.
                            nc.gpsimd.dma_start(y_ap[ts(oc, P), ts(th, NB)], o_sb[:])
                else:
                    # xstat: v1-style — stationary x token-tile, moving W bank
                    # [128, 2, 512]; timing-only build (phase="mm").
                    assert phase == "mm"
                    for ob in range(4):
                        for tt in range(8):
                            th, tl = divmod(tt, 4)
                            psum = psum_pool.tile([P, NB], f32, tag="psum", name="psum")
                            for kp in range(KP):
                                nc.tensor.matmul(
                                    psum[:],
                                    lhsT=xb[th][:, 2 * kp : 2 * kp + 2, ts(tl, P)],
                                    rhs=wb[ob][:, 2 * kp : 2 * kp + 2, :],
                                    perf_mode=mybir.MatmulPerfMode.DoubleRow,
                                    start=(kp == 0),
                                    stop=(kp == KP - 1),
                                )
                            o_sb = out_pool.tile([P, NB], fp16, tag="osb", name="o_sb")
                            nc.vector.tensor_scalar_add(
                                o_sb[:], psum[:], bias[:, 0:1]
                            )
                            # timing-only: yT is [2048, 1024]; write any
                            # distinct in-range region per (ob, tt)
                            nc.scalar.dma_start(
                                y_ap[ts(2 * ob + (tt % 2), P), ts(tt // 4, NB)],
                                o_sb[:],
                            )

            staggered = loop_mode == "staggered4"

            def body():
                for u in range(unroll):
                    if staggered and u > 0:
                        # per-copy stage boundaries: with 4 copies this gives
                        # stage == logical iteration, so the adjacent-stage
                        # rule allows exactly 1-copy prep lookahead (the
                        # auto equal split cuts mid-MM-stream and couples PE
                        # to the DMA/sign stream instead)
                        tc.stage_boundary()
                    body_one(u)

            if loop_n > 1:
                assert loop_n % unroll == 0
                with tc.For_i(
                    0,
                    loop_n // unroll,
                    1,
                    hint_engines=(mybir.EngineType.PE,),
                    staggered_reset=staggered,
                ):
                    body()
            else:
                body()

    nc.compile()
    return nc


def _get_nc():
    if "nc" not in _CACHE:
        _CACHE["nc"] = _build_bass()
    return _CACHE["nc"]


def _host_inputs(inputs):
    import ml_dtypes

    x = np.asarray(inputs["x"], dtype=np.float32)
    W = np.asarray(inputs["W"], dtype=np.float32)
    b = np.ascontiguousarray(np.asarray(inputs["b"], dtype=np.float32))

    # bf16 staging: sign-preserving (bf16 keeps fp32's exponent range);
    # layouts permuted to contraction-major [ki, ko, t|o]
    x16 = x.astype(ml_dtypes.bfloat16)
    W16 = W.astype(ml_dtypes.bfloat16)
    xp = [
        np.ascontiguousarray(
            x16[c * T_CORE : (c + 1) * T_CORE]
            .reshape(T_CORE, KO, P)
            .transpose(2, 1, 0)
        )
        for c in range(N_CORES)
    ]
    Wp = np.ascontiguousarray(W16.reshape(D_OUT, KO, P).transpose(2, 1, 0))
    # bias transposed to per-partition layout: bt[p, c] = b[c*128 + p]
    bt = np.ascontiguousarray(b.reshape(OC, P).T)
    return xp, Wp, bt


def kernel(**inputs):
    global LAST_RESULT

    from concourse.bass_utils import run_bass_kernel_spmd

    xp, Wp, bt = _host_inputs(inputs)

    nc = _get_nc()
    in_maps = [
        {"xp": xp[c], "Wp": Wp, "bt": bt}
        for c in range(N_CORES)
    ]
    res = run_bass_kernel_spmd(nc, in_maps, core_ids=list(range(N_CORES)))
    LAST_RESULT = res
    # un-transpose per-core yT [2048, 1024] -> y [1024, 2048]; widen to f32
    y = np.concatenate(
        [np.ascontiguousarray(r["yT"].T) for r in res.results], axis=0
    )
    return y.astype(np.float32)


# revision 42
# speedup vs baseline: 3.9826x; 1.1729x over previous
"""Binarized linear layer (BLinear) Trainium2 kernel, v2.

Computes y = sign(x) @ sign(W).T + b for x [8192, 2048] f32, W [2048, 2048] f32,
b [2048] f32. Data-parallel across 8 NeuronCores (1024 tokens per core, W
replicated).

Math notes (all exact => bit-exact vs the fp32 reference):
 - sign() in {-1, 0, +1} is exact in bf16/fp8e4; TensorE accumulates fp32 in
   PSUM; sums of +-1 over K=2048 are exact integers << 2^24.
 - x and W are staged to DRAM as bf16 (host cast). bf16 keeps fp32's exponent
   range, so the cast preserves sign()/zeroness for every fp32 input.
 - y is computed TRANSPOSED on device (yT [2048 o, 1024 t] fp16; integer sums
   with |y| <= 2048 are exact in fp16; bias added on-device from f32 PSUM
   before the cast) and un-transposed/widened on the host.

v2 structure (vs v1's 96.4us):
 - Stationary operand is W (lhsT = wb tile [128ki, 2ko, 128o]); each
   LDWEIGHTS is reused by 2 streaming matmuls (rhs = xb [128ki, 2ko, 512t]),
   so the DoubleRow 256-col weight load always hides under ~480ns of
   streaming.
 - The benchmark loop body holds TWO unrolled iterations and every operand
   tile pool has bufs=2, so iteration i+1's DMA-transpose + sign prep runs
   concurrently with iteration i's matmuls (cross-iteration software
   pipelining; v1 serialized prep behind the previous iteration's matmuls).
 - sign() work is split: ScalarE does most chunks natively, VectorE does 3
   W chunks via (min(v*2^126, 1) then max(.,-1)) two-op form (exact for all
   |v| >= 2^-126; inputs here have |v| >= ~2^-28).
 - PSUM eviction (bias add + fp16 cast) all on VectorE tensor_scalar_add
   with a per-partition (=per-o) bias AP.
"""

import numpy as np

N_CORES = 8
TOKENS = 8192
D_IN = 2048
D_OUT = 2048
T_CORE = TOKENS // N_CORES  # 1024 tokens per core

P = 128
KO = D_IN // P          # 16 contraction chunks of 128
KP = KO // 2            # 8 DoubleRow K-pairs (256 per matmul)
NB = 512                # matmul moving free dim / PSUM bank (fp32)
TH = T_CORE // NB       # 2 token halves of 512
OC = D_OUT // P         # 16 out-feature tiles of 128
WCH = 8                 # W prep chunks (256 o-rows each)
WCO = D_OUT // WCH      # 256 o per W chunk

_CACHE = {}
LAST_RESULT = None


def _build_bass(loop_n=1, phase="all", mm_struct="wstat", sign_gps=False,
                loop_mode="staggered8"):
    import concourse.mybir as mybir
    import concourse.tile as tile
    from concourse import bacc
    from concourse.bass import ts

    nc = bacc.Bacc(
        "TRN2",
        target_bir_lowering=False,
        debug=False,
        enable_asserts=False,
    )

    f32 = mybir.dt.float32
    bf16 = mybir.dt.bfloat16
    fp16 = mybir.dt.float16
    fp8 = mybir.dt.float8e4

    # xp/Wp are host-permuted to the contraction-major SBUF layout
    # [chunk][ki, ko, t|o] (bit-exact bf16 values, pure layout staging), so
    # each device load is one fully-contiguous-per-partition plain DMA — no
    # xbar DMA-transpose, max descriptor efficiency.
    x_d = nc.dram_tensor("xp", [TH, P, KO, NB], bf16, kind="ExternalInput")
    w_d = nc.dram_tensor("Wp", [WCH, P, KO, WCO], bf16, kind="ExternalInput")
    b_d = nc.dram_tensor("bt", [P, OC], f32, kind="ExternalInput")
    y_d = nc.dram_tensor("yT", [D_OUT, T_CORE], fp16, kind="ExternalOutput")

    x_ap = x_d.ap()
    w_ap = w_d.ap()
    b_ap = b_d.ap()
    y_ap = y_d.ap()

    if loop_n <= 1:
        unroll = 1
    elif loop_mode == "staggered8":
        unroll = 8
    else:
        unroll = 4

    with tile.TileContext(nc) as tc:
        with (
            tc.tile_pool(name="ops", bufs=2) as ops,
            tc.tile_pool(name="xstage", bufs=2) as xstage,
            tc.tile_pool(name="wstage", bufs=6) as wstage,
            tc.tile_pool(name="dvetmp", bufs=1) as dvetmp,
            tc.tile_pool(name="outp", bufs=3) as out_pool,
            tc.tile_pool(name="psum", bufs=8, space="PSUM") as psum_pool,
        ):
            ev_i = 0

            def body_one(u):
                nonlocal ev_i
                # --- operand tiles for this (unrolled) iteration ---
                xb = [ops.tile([P, KO, NB], fp8, name=f"xb{h}") for h in range(TH)]
                if mm_struct in ("wstat", "wsame"):
                    wb = [ops.tile([P, KO, WCO], fp8, name=f"wb{c}") for c in range(WCH)]
                elif mm_struct == "wswi":
                    # SwInterleave storage: per (kp, lo) a contiguous 256-elem
                    # block [A_col127, B_col127, A_col126, ..., B_col0]
                    wb = [ops.tile([P, KP, 2, 2 * P], fp8, name=f"wb{c}")
                          for c in range(WCH)]
                else:
                    wb = [ops.tile([P, KO, NB], fp8, name=f"wb{c}") for c in range(4)]
                bias = ops.tile([P, OC], f32, name="bias")

                if phase == "mm":
                    # timing-only build: tiny slice writes allocate the tiles
                    # (full contents are garbage; numerics unused)
                    for t_ in xb + wb:
                        nc.gpsimd.memset(t_[:, 0, 0, 0:1] if t_.shape()[1:] == [KP, 2, 2 * P] else t_[:, 0, 0:1], 1.0)
                    nc.gpsimd.memset(bias[:, 0:1], 0.0)
                else:
                    # --- prep: DMA-transpose from DRAM (bf16) + sign -> fp8 ---
                    nc.gpsimd.dma_start(bias[:], b_ap[:, :])

                    def sign_act(dst, src):
                        nc.scalar.sign(dst, src)

                    def sign_2op(eng, dst, src, shape):
                        # exact sign for all |v| >= 2^-126 (incl. v == 0)
                        tmp = dvetmp.tile(shape, bf16, name="dvetmp")
                        eng.tensor_scalar(
                            tmp[:], src, 2.0 ** 126, 1.0,
                            mybir.AluOpType.mult, mybir.AluOpType.min,
                        )
                        eng.tensor_scalar_max(dst, tmp[:], -1.0)

                    def prep_x(h):
                        st = xstage.tile([P, KO, NB], bf16, name="xst")
                        nc.sync.dma_start(st[:], x_ap[h])
                        if phase == "dma":
                            nc.vector.tensor_copy(xb[h][:, 0, 0:1], st[:, 0, 0:1])
                            return
                        sign_act(xb[h][:], st[:])

                    def prep_w(c):
                        st = wstage.tile([P, KO, WCO], bf16, name="wst")
                        nc.sync.dma_start(st[:], w_ap[c])
                        if phase == "dma":
                            nc.vector.tensor_copy(wb[c][:, 0, 0:1], st[:, 0, 0:1])
                            return
                        # DVE gets the LATE-consumed chunks (w5..w7): its queue
                        # drains copy u's evictions first, so u+1's DVE signs
                        # land mid-mm — early chunks must come from ACT, whose
                        # queue holds only signs and drains well before u ends.
                        if c >= 5:
                            sign_2op(nc.vector, wb[c][:], st[:], [P, KO, WCO])
                        else:
                            sign_act(wb[c][:], st[:])

                    prep_x(0)
                    prep_w(0)
                    prep_x(1)
                    for c in range(1, WCH):
                        prep_w(c)

                if phase in ("prep", "dma"):
                    # tiny consumers so prep work can't be dead-code'd away
                    o_sb = out_pool.tile([P, NB], fp16, tag="osb", name="o_sb")
                    for i, t_ in enumerate(xb + wb):
                        nc.vector.tensor_copy(
                            o_sb[:, i : i + 1], t_[:, 0, 0:1]
                        )
                    nc.scalar.dma_start(y_ap[ts(0, P), ts(0, NB)], o_sb[:])
                    return

                if mm_struct in ("wstat", "wsame"):
                    # th INNER: each stationary W tile (LDWEIGHTS) is reused by
                    # the 2 moving-x matmuls — measured ~5us/iter cheaper than
                    # reloading per MM (th-outer). In steady state prep(u+1)
                    # completes during mm(u), so x-half gating doesn't matter.
                    for oc in range(OC):
                        c, lo = divmod(oc, 2)
                        if mm_struct == "wsame":
                            c, lo = 0, 0  # fixed stationary: LDW-elision probe
                        psums = [
                            psum_pool.tile([P, NB], f32, tag="psum", name="psum")
                            for _ in range(TH)
                        ]
                        for kp in range(KP):
                            kp_ = 0 if mm_struct == "wsame" else kp
                            if mm_struct == "wswi":
                                lhsT = wb[c][:, kp_, lo, :]
                                pm = mybir.MatmulPerfMode.DoubleRowSwInterleave
                            else:
                                lhsT = wb[c][:, 2 * kp_ : 2 * kp_ + 2, ts(lo, P)]
                                pm = mybir.MatmulPerfMode.DoubleRow
                            for th in range(TH):
                                nc.tensor.matmul(
                                    psums[th][:],
                                    lhsT=lhsT,
                                    rhs=xb[th][:, 2 * kp : 2 * kp + 2, :],
                                    perf_mode=pm,
                                    start=(kp == 0),
                                    stop=(kp == KP - 1),
                                )
                        for th in range(TH):
                            o_sb = out_pool.tile([P, NB], fp16, tag="osb", name="o_sb")
                            # evictions on DVE (ACT Identity would force
                            # activation-table switches against the Sign ops)
                            nc.vector.tensor_scalar_add(
                                o_sb[:], psums[th][:], bias[:, oc : oc + 1]
                            )
                            ev_i += 1
                            # stores issue via GPS/SWDGE: the store issues are
                            # paced by the mm span, and any engine that also
                            # did sign work would FIFO-block the next copy's
                            # signs behind them. GPS does only stores + bias.
                            nc.gpsimd.dma_start(y_ap[ts(oc, P), ts(th, NB)], o_sb[:])
                else:
                    # xstat: v1-style — stationary x token-tile, moving W bank
                    # [128, 2, 512]; timing-only build (phase="mm").
                    assert phase == "mm"
                    for ob in range(4):
                        for tt in range(8):
                            th, tl = divmod(tt, 4)
                            psum = psum_pool.tile([P, NB], f32, tag="psum", name="psum")
                            for kp in range(KP):
                                nc.tensor.matmul(
                                    psum[:],
                                    lhsT=xb[th][:, 2 * kp : 2 * kp + 2, ts(tl, P)],
                                    rhs=wb[ob][:, 2 * kp : 2 * kp + 2, :],
                                    perf_mode=mybir.MatmulPerfMode.DoubleRow,
                                    start=(kp == 0),
                                    stop=(kp == KP - 1),
                                )
                            o_sb = out_pool.tile([P, NB], fp16, tag="osb", name="o_sb")
                            nc.vector.tensor_scalar_add(
                                o_sb[:], psum[:], bias[:, 0:1]
                            )
                            # timing-only: yT is [2048, 1024]; write any
                            # distinct in-range region per (ob, tt)
                            nc.scalar.dma_start(
                                y_ap[ts(2 * ob + (tt % 2), P), ts(tt // 4, NB)],
                                o_sb[:],
                            )

            staggered = loop_mode in ("staggered4", "staggered8")
            copies_per_stage = unroll // 4 if staggered else unroll

            def body():
                for u in range(unroll):
                    if staggered and u > 0 and u % copies_per_stage == 0:
                        # per-copy-group stage boundaries: stage == 1-2 logical
                        # iterations, so the adjacent-stage rule allows 1-3
                        # copies of prep lookahead (the auto equal split cuts
                        # mid-MM-stream and couples PE to the DMA/sign stream)
                        tc.stage_boundary()
                    body_one(u)

            if loop_n > 1:
                assert loop_n % unroll == 0
                with tc.For_i(
                    0,
                    loop_n // unroll,
                    1,
                    hint_engines=(mybir.EngineType.PE,),
                    staggered_reset=staggered,
                ):
                    body()
            else:
                body()

    nc.compile()
    return nc


def _get_nc():
    if "nc" not in _CACHE:
        _CACHE["nc"] = _build_bass()
    return _CACHE["nc"]


def _host_inputs(inputs):
    import ml_dtypes

    x = np.asarray(inputs["x"], dtype=np.float32)
    W = np.asarray(inputs["W"], dtype=np.float32)
    b = np.ascontiguousarray(np.asarray(inputs["b"], dtype=np.float32))

    # bf16 staging: sign-preserving (bf16 keeps fp32's exponent range);
    # layouts permuted to contraction-major [ki, ko, t|o]
    x16 = x.astype(ml_dtypes.bfloat16)
    W16 = W.astype(ml_dtypes.bfloat16)
    xp = [
        np.ascontiguousarray(
            x16[c * T_CORE : (c + 1) * T_CORE]
            .reshape(TH, NB, KO, P)
            .transpose(0, 3, 2, 1)
        )
        for c in range(N_CORES)
    ]
    Wp = np.ascontiguousarray(
        W16.reshape(WCH, WCO, KO, P).transpose(0, 3, 2, 1)
    )
    # bias transposed to per-partition layout: bt[p, c] = b[c*128 + p]
    bt = np.ascontiguousarray(b.reshape(OC, P).T)
    return xp, Wp, bt


def kernel(**inputs):
    global LAST_RESULT

    from concourse.bass_utils import run_bass_kernel_spmd

    xp, Wp, bt = _host_inputs(inputs)

    nc = _get_nc()
    in_maps = [
        {"xp": xp[c], "Wp": Wp, "bt": bt}
        for c in range(N_CORES)
    ]
    res = run_bass_kernel_spmd(nc, in_maps, core_ids=list(range(N_CORES)))
    LAST_RESULT = res
    # un-transpose per-core yT [2048, 1024] -> y [1024, 2048]; widen to f32
    y = np.concatenate(
        [np.ascontiguousarray(r["yT"].T) for r in res.results], axis=0
    )
    return y.astype(np.float32)
